# revision 27
# baseline (speedup 1.0000x reference)
"""Causal self-attention (GQA, partial RoPE, qk rms-norm, logit softcap) on 8 trn2 cores.

Sharding: 8 cores = batch(2) x kv_head(4). Each core computes, for its (b, h):
  - q/k/v projections for its 4 q-heads / 1 kv-head (x @ W.T slices)
  - rms-norm, partial rope, q_gain, causal softcapped attention
  - partial output projection against Wproj columns [512h:512h+512]
Host sums the 4 partials per batch.

Optimizations vs the fp32 baseline (438us -> ~307us):
  - All DRAM traffic fp16 (x, wqkv, wpT, cos/sin, out partials), host-side
    pre-swizzled so every DMA is contiguous; the startup weight stream is
    interleaved with the first x tiles and rope tables.
  - Matmul operands stored 16-bit in SBUF (qT/kT/v/yT fp16, probabilities
    bf16 for exp range) to halve PE SBUF read bandwidth.
  - Softmax denominator: vector pre-sums each pair of probability blocks
    (p0+p1, bf16); the PE accumulates ones[128,128] @ pairsum into a PSUM
    bank per head, which reduces over partitions AND broadcasts in one shot
    (half the columns of a per-block ones matmul, no bank-to-bank copies);
    reciprocal via the fast approximate DVE op.
  - Attention is software-pipelined by one k-pair (AV + denominator matmuls
    of pair j-1 emitted after the scores of pair j) and one out-proj chain of
    the previous chunk is inserted after each scores pair, covering the
    scalar tanh+exp latency without PE stalls.
Note: fusing phase 1 into the attention stream (keeping all engines busy
simultaneously) was tried and is SLOWER (387us) — the sustained multi-engine
activity trips the power throttle (activity_1 util limit 0.5 -> PE at
1.2GHz instead of 2.4GHz). The phase-separated schedule wins.
"""
import math
import numpy as np
from contextlib import ExitStack

import concourse.bass as bass
import concourse.tile as tile
from concourse import bacc, mybir
from concourse.bass_utils import run_bass_kernel_spmd
from concourse.masks import make_identity
from concourse.alu_op_type import AluOpType

F32 = mybir.dt.float32
F32R = mybir.dt.float32r
F16 = mybir.dt.float16
BF16 = mybir.dt.bfloat16

B = 2
S = 2048
D = 2048
H = 16
HKV = 4
HD = 128
G = 4  # q heads per kv head (= heads per core)
ROPE = 32
HALF = ROPE // 2  # 16
ROPE_BASE = 10000.0
CAP = 30.0
EPS = float(np.finfo(np.float32).eps)
NST = S // 128  # 16 s-tiles
NCH = S // 512  # 4 sq chunks
NDT = D // 128  # 16 d k-tiles
FQKV = G * HD + 2 * HD  # 768

_CACHE = {}


def _build():
    nc = bacc.Bacc("TRN2", target_bir_lowering=False, debug=False)

    # all pre-swizzled host-side so each DMA is contiguous per partition
    xsw = nc.dram_tensor("xsw", [NST, 128, NDT * 128], F16, kind="ExternalInput").ap()
    wqkv = nc.dram_tensor("wqkv", [128, NDT * FQKV], F16, kind="ExternalInput").ap()
    wpT = nc.dram_tensor("wpT", [128, G * D], F16, kind="ExternalInput").ap()
    gains = nc.dram_tensor("gains", [128, G], F32, kind="ExternalInput").ap()
    cos4 = nc.dram_tensor("cos4", [128, NST * G * HALF], F16, kind="ExternalInput").ap()
    sin4 = nc.dram_tensor("sin4", [128, NST * G * HALF], F16, kind="ExternalInput").ap()
    osw = nc.dram_tensor("osw", [NST, NCH, 128, 512], F16, kind="ExternalOutput").ap()

    with tile.TileContext(nc) as tc:
        with ExitStack() as ctx:
            persist = ctx.enter_context(tc.tile_pool(name="persist", bufs=1))

            # ---- persistent intermediates ----
            qT_all = persist.tile([128, G, S], F16)    # [f, g, s]
            kT_all = persist.tile([128, S], F16)       # [f, s]
            v_all = persist.tile([128, NST, HD], F16)  # [sk within tile, st, f]
            yT_all = persist.tile([128, G, S], F16)    # [f, g, s]

            # ---- constants ----
            ident = persist.tile([128, 128], F32)
            make_identity(nc, ident)

            ones_f = persist.tile([128, 128], F32)
            nc.vector.memset(ones_f, 1.0)
            ones128 = persist.tile([128, 128], BF16)  # reduce+broadcast stationary
            nc.vector.tensor_copy(ones128, ones_f)

            # diagonal-block 0/1 masks (r = kb - 4c in 0..3): valid iff sq >= r*128 + sk
            masks = persist.tile([128, 4, 512], BF16)
            mask_f = persist.tile([128, 512], F32)
            for r in range(4):
                nc.vector.memset(mask_f, 1.0)
                nc.gpsimd.affine_select(
                    out=mask_f, in_=mask_f, compare_op=AluOpType.is_ge,
                    fill=0.0, base=-128 * r, pattern=[[1, 512]], channel_multiplier=-1,
                )
                nc.vector.tensor_copy(masks[:, r, :], mask_f)

            eps_t = persist.tile([128, 1], F32)
            nc.vector.memset(eps_t, EPS)

            gains_sb = persist.tile([128, G], F32)
            cos_all = persist.tile([128, NST, G * HALF], F16)
            sin_all = persist.tile([128, NST, G * HALF], F16)

            # ================= phase 1: qkv projection + rms + rope + transposes ====
            with (
                tc.tile_pool(name="wq", bufs=1) as wq_pool,
                tc.tile_pool(name="xc", bufs=4) as xc_pool,
                tc.tile_pool(name="p1w", bufs=3) as p1w,
                tc.tile_pool(name="p1s", bufs=4) as p1s,
                tc.tile_pool(name="psq", bufs=3, space="PSUM") as psq_pool,
                tc.tile_pool(name="pskv", bufs=2, space="PSUM") as pskv_pool,
                tc.tile_pool(name="pstr", bufs=3, space="PSUM") as pstr_pool,
            ):
                # x tiles for the first s-tiles and the rope tables are
                # interleaved with the weight stream so neither the first
                # matmul chain nor s-tile 0's rope wait on the full 3MB of
                # wqkv to land
                # quarter-granular first x tile: the st-0 matmul chain's dep on
                # xc0 is per-slice, so the PE starts after ~128KB + one weight
                # slice instead of the full 512KB + weights
                xc0 = xc_pool.tile([128, NDT, 128], F16)
                xsw0 = xsw[0].rearrange("p (dt s) -> p dt s", dt=NDT)
                wqkv_sb = wq_pool.tile([128, NDT, FQKV], F16)
                nc.sync.dma_start(out=xc0[:, 0:4, :], in_=xsw0[:, 0:4, :])
                nc.sync.dma_start(out=wqkv_sb[:, 0, :], in_=wqkv[:, 0:FQKV])
                nc.sync.dma_start(out=xc0[:, 4:8, :], in_=xsw0[:, 4:8, :])
                for dt in range(1, 4):
                    nc.sync.dma_start(out=wqkv_sb[:, dt, :],
                                      in_=wqkv[:, dt * FQKV:(dt + 1) * FQKV])
                nc.sync.dma_start(out=xc0[:, 8:16, :], in_=xsw0[:, 8:16, :])
                nc.sync.dma_start(out=gains_sb, in_=gains)
                nc.sync.dma_start(out=cos_all,
                                  in_=cos4.rearrange("p (t f) -> p t f", t=NST))
                nc.sync.dma_start(out=sin_all,
                                  in_=sin4.rearrange("p (t f) -> p t f", t=NST))
                xc1 = xc_pool.tile([128, NDT, 128], F16)
                nc.sync.dma_start(out=xc1, in_=xsw[1].rearrange("p (dt s) -> p dt s", dt=NDT))
                for dt in range(4, NDT):
                    nc.sync.dma_start(out=wqkv_sb[:, dt, :],
                                      in_=wqkv[:, dt * FQKV:(dt + 1) * FQKV])

                for st in range(NST):
                    if st == 0:
                        xc = xc0
                    elif st == 1:
                        xc = xc1
                    else:
                        xc = xc_pool.tile([128, NDT, 128], F16)
                        nc.sync.dma_start(
                            out=xc, in_=xsw[st].rearrange("p (dt s) -> p dt s", dt=NDT))

                    psq = psq_pool.tile([128, G * HD], F32)
                    pskv = pskv_pool.tile([128, 2 * HD], F32)
                    for dt in range(NDT):
                        nc.tensor.matmul(psq, xc[:, dt, :], wqkv_sb[:, dt, 0:G * HD],
                                         start=(dt == 0), stop=(dt == NDT - 1))
                    for dt in range(NDT):
                        nc.tensor.matmul(pskv, xc[:, dt, :], wqkv_sb[:, dt, G * HD:FQKV],
                                         start=(dt == 0), stop=(dt == NDT - 1))

                    # v: straight evacuation (no norm) — on ScalarE
                    nc.scalar.copy(v_all[:, st, :], pskv[:, HD:2 * HD])

                    # rms statistics for q (4 heads) and k
                    q2 = p1w.tile([128, G * HD], F32, tag="q2")
                    k2 = p1w.tile([128, HD], F32, tag="k2")
                    nc.scalar.activation(q2, psq, mybir.ActivationFunctionType.Square)
                    nc.scalar.activation(k2, pskv[:, 0:HD], mybir.ActivationFunctionType.Square)
                    ms = p1s.tile([128, 5], F32, tag="ms")
                    nc.vector.reduce_sum(ms[:, 0:4], q2.rearrange("p (g d) -> p g d", g=G),
                                         axis=mybir.AxisListType.X)
                    nc.vector.reduce_sum(ms[:, 4:5], k2, axis=mybir.AxisListType.X)
                    rstd = p1s.tile([128, 5], F32, tag="rstd")
                    nc.scalar.activation(rstd, ms, mybir.ActivationFunctionType.Sqrt,
                                         scale=1.0 / HD, bias=eps_t)
                    nc.vector.reciprocal(rstd, rstd)
                    gsc = p1s.tile([128, G], F32, tag="gsc")
                    nc.vector.tensor_mul(gsc, rstd[:, 0:4], gains_sb)

                    # rope rotation (reads PSUM directly), then pass-through copy
                    cos_t = cos_all[:, st, :].rearrange("p (g d) -> p g d", g=G)
                    sin_t = sin_all[:, st, :].rearrange("p (g d) -> p g d", g=G)
                    psq_v = psq.rearrange("p (g d) -> p g d", g=G)

                    q_rot = p1w.tile([128, G, HD], F32, tag="q_rot")
                    tmp = p1w.tile([128, G, HALF], F32, tag="tmp")
                    nc.vector.tensor_mul(q_rot[:, :, 0:HALF], psq_v[:, :, 0:HALF], cos_t)
                    nc.vector.tensor_mul(tmp, psq_v[:, :, HALF:ROPE], sin_t)
                    nc.vector.tensor_add(q_rot[:, :, 0:HALF], q_rot[:, :, 0:HALF], tmp)
                    nc.vector.tensor_mul(q_rot[:, :, HALF:ROPE], psq_v[:, :, HALF:ROPE], cos_t)
                    nc.vector.tensor_mul(tmp, psq_v[:, :, 0:HALF], sin_t)
                    nc.vector.tensor_sub(q_rot[:, :, HALF:ROPE], q_rot[:, :, HALF:ROPE], tmp)
                    nc.vector.tensor_copy(q_rot[:, :, ROPE:HD], psq_v[:, :, ROPE:HD])

                    k_rot = p1w.tile([128, HD], F32, tag="k_rot")
                    ktmp = p1w.tile([128, HALF], F32, tag="ktmp")
                    kc = cos_all[:, st, 0:HALF]
                    ks = sin_all[:, st, 0:HALF]
                    nc.vector.tensor_mul(k_rot[:, 0:HALF], pskv[:, 0:HALF], kc)
                    nc.vector.tensor_mul(ktmp, pskv[:, HALF:ROPE], ks)
                    nc.vector.tensor_add(k_rot[:, 0:HALF], k_rot[:, 0:HALF], ktmp)
                    nc.vector.tensor_mul(k_rot[:, HALF:ROPE], pskv[:, HALF:ROPE], kc)
                    nc.vector.tensor_mul(ktmp, pskv[:, 0:HALF], ks)
                    nc.vector.tensor_sub(k_rot[:, HALF:ROPE], k_rot[:, HALF:ROPE], ktmp)
                    nc.vector.tensor_copy(k_rot[:, ROPE:HD], pskv[:, ROPE:HD])

                    # scale in place: q by gain*rstd/sqrt(hd), k by rstd
                    for h in range(G):
                        nc.vector.tensor_scalar_mul(q_rot[:, h, :], q_rot[:, h, :], gsc[:, h:h + 1])
                    nc.vector.tensor_scalar_mul(k_rot, k_rot, rstd[:, 4:5])

                    # transposes into [f, s] layouts; evacuations (fp32->fp16) on ScalarE
                    for h in range(G):
                        ptr = pstr_pool.tile([128, 128], F32)
                        nc.tensor.transpose(ptr, q_rot[:, h, :], ident)
                        nc.scalar.copy(qT_all[:, h, st * 128:(st + 1) * 128], ptr)
                    ptr = pstr_pool.tile([128, 128], F32)
                    nc.tensor.transpose(ptr, k_rot, ident)
                    nc.scalar.copy(kT_all[:, st * 128:(st + 1) * 128], ptr)

            # ======== phase 2+3: attention with interleaved output projection ======
            # Attention is software-pipelined by one k-pair: the PE queue per
            # head is [scores(j), AV(j-1), ones(j-1), scores(j+1), ...] with an
            # out-proj chain of the PREVIOUS chunk dropped in after each scores
            # pair, so the PE always has independent work while the scalar
            # engine runs tanh+exp for the freshly computed pair.
            # The softmax denominator costs the PE one ones128 matmul per PAIR
            # (vector pre-sums p0+p1), accumulated+broadcast in a PSUM bank.
            with (
                tc.tile_pool(name="wp", bufs=1) as wp_pool,
                tc.tile_pool(name="p2s", bufs=4) as p2s,
                tc.tile_pool(name="p2n", bufs=2) as p2n,
                tc.tile_pool(name="p3s", bufs=3) as p3s,
                tc.tile_pool(name="pss", bufs=2, space="PSUM") as pss_pool,
                tc.tile_pool(name="psy", bufs=1, space="PSUM") as psy_pool,
                tc.tile_pool(name="psb", bufs=1, space="PSUM") as psb_pool,
                tc.tile_pool(name="pso", bufs=2, space="PSUM") as pso_pool,
            ):
                wpT_sb = wp_pool.tile([128, G, D], F16)
                nc.sync.dma_start(out=wpT_sb,
                                  in_=wpT.rearrange("p (g j) -> p g j", g=G))

                pending = []  # out-proj (st, jc) chains left to emit

                def emit_outproj(n):
                    for _ in range(min(n, len(pending))):
                        st, jc = pending.pop(0)
                        ps_o = pso_pool.tile([128, 512], F32)
                        for g in range(G):
                            nc.tensor.matmul(
                                ps_o,
                                yT_all[:, g, st * 128:(st + 1) * 128],
                                wpT_sb[:, g, jc * 512:(jc + 1) * 512],
                                start=(g == 0), stop=(g == G - 1),
                            )
                        o_sb = p3s.tile([128, 512], F16)
                        nc.vector.tensor_copy(o_sb, ps_o)
                        nc.sync.dma_start(out=osw[st][jc], in_=o_sb)

                def attention_head(c, g):
                    nkv = 4 * (c + 1)
                    npair = nkv // 2
                    qT_c = qT_all[:, g, c * 512:(c + 1) * 512]
                    ps_y = psy_pool.tile([128, 512], F32)
                    ps_b = psb_pool.tile([128, 512], F32)
                    deferred = None  # (kb0, off, p_tile, psum_tile_of_pairsum)

                    def emit_deferred():
                        kb0, off, p, ps01 = deferred
                        for i in range(2):
                            kb = kb0 + i
                            nc.tensor.matmul(ps_y[:, off:512], v_all[:, kb, :],
                                             p[:, i, off:512],
                                             start=(kb == 0), stop=(kb == nkv - 1))
                        j = kb0 // 2
                        nc.tensor.matmul(ps_b[:, off:512], ones128, ps01[:, off:512],
                                         start=(j == 0), stop=(j == npair - 1))

                    for j, kb0 in enumerate(range(0, nkv, 2)):
                        # the r>=2 diagonal pair only touches sq >= 256
                        off = 256 if kb0 - 4 * c == 2 else 0
                        ps_s = pss_pool.tile([128, 2, 512], F32, tag="ps_s")
                        for i in range(2):
                            kb = kb0 + i
                            nc.tensor.matmul(
                                ps_s[:, i, off:512],
                                kT_all[:, kb * 128:(kb + 1) * 128], qT_c[:, off:512],
                                start=True, stop=True,
                            )
                        emit_outproj(1)
                        if deferred is not None:
                            emit_deferred()
                        t = p2s.tile([128, 2, 512], F32, tag="t")
                        nc.scalar.activation(t[:, :, off:512], ps_s[:, :, off:512],
                                             mybir.ActivationFunctionType.Tanh,
                                             scale=1.0 / CAP)
                        p = p2s.tile([128, 2, 512], BF16, tag="p")
                        nc.scalar.activation(p[:, :, off:512], t[:, :, off:512],
                                             mybir.ActivationFunctionType.Exp,
                                             scale=CAP)
                        for i in range(2):
                            r = kb0 + i - 4 * c
                            if r >= 0:
                                nc.vector.tensor_mul(p[:, i, off:512], p[:, i, off:512],
                                                     masks[:, r, off:512])
                        ps01 = p2s.tile([128, 512], BF16, tag="ps01")
                        nc.vector.tensor_add(ps01[:, off:512], p[:, 0, off:512],
                                             p[:, 1, off:512])
                        deferred = (kb0, off, p, ps01)
                    emit_deferred()
                    # ps_b holds the denominator broadcast to all partitions
                    recip = p2n.tile([128, 512], F32, tag="recip")
                    nc.vector.reciprocal_approx_fast(recip, ps_b)
                    nc.vector.tensor_mul(yT_all[:, g, c * 512:(c + 1) * 512], ps_y, recip)

                for c in range(NCH):
                    for g in range(G):
                        attention_head(c, g)
                    pending.extend((st, jc)
                                   for st in range(4 * c, 4 * c + 4)
                                   for jc in range(4))
                emit_outproj(len(pending))

    nc.compile()
    return nc


def _host_prep(x, Wq, Wk, Wv, Wproj, q_gain):
    inv_freq = 1.0 / (ROPE_BASE ** (np.arange(0, ROPE, 2, dtype=np.float32) / ROPE))
    t = np.arange(S, dtype=np.float32)
    freqs = np.outer(t, inv_freq).astype(np.float32)  # [S, 16]
    cos = np.cos(freqs)
    sin = np.sin(freqs)
    # [S, G*HALF] -> swizzle (t p) f -> [128, NST * G*HALF]
    cos4 = np.tile(cos[:, None, :], (1, G, 1)).reshape(NST, 128, G * HALF)
    sin4 = np.tile(sin[:, None, :], (1, G, 1)).reshape(NST, 128, G * HALF)
    cos4 = np.ascontiguousarray(cos4.transpose(1, 0, 2).reshape(128, NST * G * HALF)).astype(np.float16)
    sin4 = np.ascontiguousarray(sin4.transpose(1, 0, 2).reshape(128, NST * G * HALF)).astype(np.float16)

    # x: [B, S, D] -> xT [D, S] -> [st, p, dt*128] where row d = dt*128 + p,
    # col s = st*128 + s'
    xsw = []
    for b in range(B):
        xT = x[b].T.reshape(NDT, 128, NST, 128)          # [dt, p, st, s']
        xsw.append(np.ascontiguousarray(
            xT.transpose(2, 1, 0, 3).reshape(NST, 128, NDT * 128)).astype(np.float16))

    in_maps = []
    for core in range(8):
        b, h = core // HKV, core % HKV
        wqkv_cat = np.concatenate(
            [Wq[512 * h:512 * h + 512].T,
             Wk[128 * h:128 * h + 128].T,
             Wv[128 * h:128 * h + 128].T], axis=1
        )                                                 # [D, FQKV]
        wqkv_sw = np.ascontiguousarray(
            wqkv_cat.reshape(NDT, 128, FQKV).transpose(1, 0, 2).reshape(128, NDT * FQKV)
        ).astype(np.float16)
        wpT = Wproj[:, 512 * h:512 * h + 512].T           # [512, D]
        wpT_sw = np.ascontiguousarray(
            wpT.reshape(G, 128, D).transpose(1, 0, 2).reshape(128, G * D)
        ).astype(np.float16)
        gains = np.ascontiguousarray(
            np.broadcast_to((q_gain[G * h:G * h + G] / math.sqrt(HD)).astype(np.float32)[None, :],
                            (128, G))
        )
        in_maps.append({
            "xsw": xsw[b],
            "wqkv": wqkv_sw,
            "wpT": wpT_sw,
            "gains": gains,
            "cos4": cos4,
            "sin4": sin4,
        })
    return in_maps


def kernel(x, Wq, Wk, Wv, Wproj, q_gain, _trace=False):
    x = np.asarray(x, dtype=np.float32)
    Wq = np.asarray(Wq, dtype=np.float32)
    Wk = np.asarray(Wk, dtype=np.float32)
    Wv = np.asarray(Wv, dtype=np.float32)
    Wproj = np.asarray(Wproj, dtype=np.float32)
    q_gain = np.asarray(q_gain, dtype=np.float32)

    if "nc" not in _CACHE:
        _CACHE["nc"] = _build()
    nc = _CACHE["nc"]

    in_maps = _host_prep(x, Wq, Wk, Wv, Wproj, q_gain)
    res = run_bass_kernel_spmd(nc, in_maps, core_ids=list(range(8)), trace=_trace)

    out = np.empty((B, S, D), dtype=np.float32)
    for b in range(B):
        acc = np.zeros((NST, NCH, 128, 512), dtype=np.float32)
        for h in range(HKV):
            acc += res.results[b * HKV + h]["osw"]
        # [st, jc, p, n] -> [st*128+p, jc*512+n]
        out[b] = acc.transpose(0, 2, 1, 3).reshape(S, D)
    if _trace:
        return out, res
    return out


# revision 28
# speedup vs baseline: 1.0055x; 1.0055x over previous
"""Causal self-attention (GQA, partial RoPE, qk rms-norm, logit softcap) on 8 trn2 cores.

Sharding: 8 cores = batch(2) x kv_head(4). Each core computes, for its (b, h):
  - q/k/v projections for its 4 q-heads / 1 kv-head (x @ W.T slices)
  - rms-norm, partial rope, q_gain, causal softcapped attention
  - partial output projection against Wproj columns [512h:512h+512]
Host sums the 4 partials per batch.

Optimizations vs the fp32 baseline (438us -> ~307us):
  - All DRAM traffic fp16 (x, wqkv, wpT, cos/sin, out partials), host-side
    pre-swizzled so every DMA is contiguous; the startup weight stream is
    interleaved with the first x tiles and rope tables.
  - Matmul operands stored 16-bit in SBUF (qT/kT/v/yT fp16, probabilities
    bf16 for exp range) to halve PE SBUF read bandwidth.
  - Softmax denominator: vector pre-sums each pair of probability blocks
    (p0+p1, bf16); the PE accumulates ones[128,128] @ pairsum into a PSUM
    bank per head, which reduces over partitions AND broadcasts in one shot
    (half the columns of a per-block ones matmul, no bank-to-bank copies);
    reciprocal via the fast approximate DVE op.
  - Attention is software-pipelined by one k-pair (AV + denominator matmuls
    of pair j-1 emitted after the scores of pair j) and one out-proj chain of
    the previous chunk is inserted after each scores pair, covering the
    scalar tanh+exp latency without PE stalls.
Note: fusing phase 1 into the attention stream (keeping all engines busy
simultaneously) was tried and is SLOWER (387us) — the sustained multi-engine
activity trips the power throttle (activity_1 util limit 0.5 -> PE at
1.2GHz instead of 2.4GHz). The phase-separated schedule wins.
"""
import math
import numpy as np
from contextlib import ExitStack

import concourse.bass as bass
import concourse.tile as tile
from concourse import bacc, mybir
from concourse.bass_utils import run_bass_kernel_spmd
from concourse.masks import make_identity
from concourse.alu_op_type import AluOpType

F32 = mybir.dt.float32
F32R = mybir.dt.float32r
F16 = mybir.dt.float16
BF16 = mybir.dt.bfloat16

B = 2
S = 2048
D = 2048
H = 16
HKV = 4
HD = 128
G = 4  # q heads per kv head (= heads per core)
ROPE = 32
HALF = ROPE // 2  # 16
ROPE_BASE = 10000.0
CAP = 30.0
EPS = float(np.finfo(np.float32).eps)
NST = S // 128  # 16 s-tiles
NCH = S // 512  # 4 sq chunks
NDT = D // 128  # 16 d k-tiles
FQKV = G * HD + 2 * HD  # 768

_CACHE = {}


def _build():
    nc = bacc.Bacc("TRN2", target_bir_lowering=False, debug=False)

    # all pre-swizzled host-side so each DMA is contiguous per partition
    xsw = nc.dram_tensor("xsw", [NST, 128, NDT * 128], F16, kind="ExternalInput").ap()
    wqkv = nc.dram_tensor("wqkv", [128, NDT * FQKV], F16, kind="ExternalInput").ap()
    wpT = nc.dram_tensor("wpT", [128, G * D], F16, kind="ExternalInput").ap()
    gains = nc.dram_tensor("gains", [128, G], F32, kind="ExternalInput").ap()
    cos4 = nc.dram_tensor("cos4", [128, NST * G * HALF], F16, kind="ExternalInput").ap()
    sin4 = nc.dram_tensor("sin4", [128, NST * G * HALF], F16, kind="ExternalInput").ap()
    osw = nc.dram_tensor("osw", [NST, NCH, 128, 512], F16, kind="ExternalOutput").ap()

    with tile.TileContext(nc) as tc:
        with ExitStack() as ctx:
            persist = ctx.enter_context(tc.tile_pool(name="persist", bufs=1))

            # ---- persistent intermediates ----
            qT_all = persist.tile([128, G, S], F16)    # [f, g, s]
            kT_all = persist.tile([128, S], F16)       # [f, s]
            v_all = persist.tile([128, NST, HD], F16)  # [sk within tile, st, f]
            yT_all = persist.tile([128, G, S], F16)    # [f, g, s]

            # ---- constants ----
            ident = persist.tile([128, 128], F32)
            make_identity(nc, ident)

            ones_f = persist.tile([128, 128], F32)
            nc.vector.memset(ones_f, 1.0)
            ones128 = persist.tile([128, 128], BF16)  # reduce+broadcast stationary
            nc.vector.tensor_copy(ones128, ones_f)

            # diagonal-block 0/1 masks (r = kb - 4c in 0..3): valid iff sq >= r*128 + sk
            masks = persist.tile([128, 4, 512], BF16)
            mask_f = persist.tile([128, 512], F32)
            for r in range(4):
                nc.vector.memset(mask_f, 1.0)
                nc.gpsimd.affine_select(
                    out=mask_f, in_=mask_f, compare_op=AluOpType.is_ge,
                    fill=0.0, base=-128 * r, pattern=[[1, 512]], channel_multiplier=-1,
                )
                nc.vector.tensor_copy(masks[:, r, :], mask_f)

            eps_t = persist.tile([128, 1], F32)
            nc.vector.memset(eps_t, EPS)

            gains_sb = persist.tile([128, G], F32)
            cos_all = persist.tile([128, NST, G * HALF], F16)
            sin_all = persist.tile([128, NST, G * HALF], F16)

            # ================= phase 1: qkv projection + rms + rope + transposes ====
            with (
                tc.tile_pool(name="wq", bufs=1) as wq_pool,
                tc.tile_pool(name="xc", bufs=4) as xc_pool,
                tc.tile_pool(name="p1w", bufs=4) as p1w,
                tc.tile_pool(name="p1s", bufs=4) as p1s,
                tc.tile_pool(name="psq", bufs=3, space="PSUM") as psq_pool,
                tc.tile_pool(name="pskv", bufs=2, space="PSUM") as pskv_pool,
                tc.tile_pool(name="pstr", bufs=3, space="PSUM") as pstr_pool,
            ):
                # x tiles for the first s-tiles and the rope tables are
                # interleaved with the weight stream so neither the first
                # matmul chain nor s-tile 0's rope wait on the full 3MB of
                # wqkv to land
                # quarter-granular first x tile: the st-0 matmul chain's dep on
                # xc0 is per-slice, so the PE starts after ~128KB + one weight
                # slice instead of the full 512KB + weights
                xc0 = xc_pool.tile([128, NDT, 128], F16)
                xsw0 = xsw[0].rearrange("p (dt s) -> p dt s", dt=NDT)
                wqkv_sb = wq_pool.tile([128, NDT, FQKV], F16)
                nc.sync.dma_start(out=xc0[:, 0:4, :], in_=xsw0[:, 0:4, :])
                nc.sync.dma_start(out=wqkv_sb[:, 0, :], in_=wqkv[:, 0:FQKV])
                nc.sync.dma_start(out=xc0[:, 4:8, :], in_=xsw0[:, 4:8, :])
                for dt in range(1, 4):
                    nc.sync.dma_start(out=wqkv_sb[:, dt, :],
                                      in_=wqkv[:, dt * FQKV:(dt + 1) * FQKV])
                nc.sync.dma_start(out=xc0[:, 8:16, :], in_=xsw0[:, 8:16, :])
                nc.sync.dma_start(out=gains_sb, in_=gains)
                nc.sync.dma_start(out=cos_all,
                                  in_=cos4.rearrange("p (t f) -> p t f", t=NST))
                nc.sync.dma_start(out=sin_all,
                                  in_=sin4.rearrange("p (t f) -> p t f", t=NST))
                xc1 = xc_pool.tile([128, NDT, 128], F16)
                nc.sync.dma_start(out=xc1, in_=xsw[1].rearrange("p (dt s) -> p dt s", dt=NDT))
                for dt in range(4, NDT):
                    nc.sync.dma_start(out=wqkv_sb[:, dt, :],
                                      in_=wqkv[:, dt * FQKV:(dt + 1) * FQKV])

                for st in range(NST):
                    if st == 0:
                        xc = xc0
                    elif st == 1:
                        xc = xc1
                    else:
                        xc = xc_pool.tile([128, NDT, 128], F16)
                        nc.sync.dma_start(
                            out=xc, in_=xsw[st].rearrange("p (dt s) -> p dt s", dt=NDT))

                    psq = psq_pool.tile([128, G * HD], F32)
                    pskv = pskv_pool.tile([128, 2 * HD], F32)
                    for dt in range(NDT):
                        nc.tensor.matmul(psq, xc[:, dt, :], wqkv_sb[:, dt, 0:G * HD],
                                         start=(dt == 0), stop=(dt == NDT - 1))
                    for dt in range(NDT):
                        nc.tensor.matmul(pskv, xc[:, dt, :], wqkv_sb[:, dt, G * HD:FQKV],
                                         start=(dt == 0), stop=(dt == NDT - 1))

                    # v: straight evacuation (no norm) — on ScalarE
                    nc.scalar.copy(v_all[:, st, :], pskv[:, HD:2 * HD])

                    # rms statistics for q (4 heads) and k
                    q2 = p1w.tile([128, G * HD], F32, tag="q2")
                    k2 = p1w.tile([128, HD], F32, tag="k2")
                    nc.scalar.activation(q2, psq, mybir.ActivationFunctionType.Square)
                    nc.scalar.activation(k2, pskv[:, 0:HD], mybir.ActivationFunctionType.Square)
                    ms = p1s.tile([128, 5], F32, tag="ms")
                    nc.vector.reduce_sum(ms[:, 0:4], q2.rearrange("p (g d) -> p g d", g=G),
                                         axis=mybir.AxisListType.X)
                    nc.vector.reduce_sum(ms[:, 4:5], k2, axis=mybir.AxisListType.X)
                    rstd = p1s.tile([128, 5], F32, tag="rstd")
                    nc.scalar.activation(rstd, ms, mybir.ActivationFunctionType.Sqrt,
                                         scale=1.0 / HD, bias=eps_t)
                    nc.vector.reciprocal(rstd, rstd)
                    gsc = p1s.tile([128, G], F32, tag="gsc")
                    nc.vector.tensor_mul(gsc, rstd[:, 0:4], gains_sb)

                    # rope rotation (reads PSUM directly), then pass-through copy
                    cos_t = cos_all[:, st, :].rearrange("p (g d) -> p g d", g=G)
                    sin_t = sin_all[:, st, :].rearrange("p (g d) -> p g d", g=G)
                    psq_v = psq.rearrange("p (g d) -> p g d", g=G)

                    q_rot = p1w.tile([128, G, HD], F32, tag="q_rot")
                    tmp = p1w.tile([128, G, HALF], F32, tag="tmp")
                    nc.vector.tensor_mul(q_rot[:, :, 0:HALF], psq_v[:, :, 0:HALF], cos_t)
                    nc.vector.tensor_mul(tmp, psq_v[:, :, HALF:ROPE], sin_t)
                    nc.vector.tensor_add(q_rot[:, :, 0:HALF], q_rot[:, :, 0:HALF], tmp)
                    nc.vector.tensor_mul(q_rot[:, :, HALF:ROPE], psq_v[:, :, HALF:ROPE], cos_t)
                    nc.vector.tensor_mul(tmp, psq_v[:, :, 0:HALF], sin_t)
                    nc.vector.tensor_sub(q_rot[:, :, HALF:ROPE], q_rot[:, :, HALF:ROPE], tmp)
                    nc.vector.tensor_copy(q_rot[:, :, ROPE:HD], psq_v[:, :, ROPE:HD])

                    k_rot = p1w.tile([128, HD], F32, tag="k_rot")
                    ktmp = p1w.tile([128, HALF], F32, tag="ktmp")
                    kc = cos_all[:, st, 0:HALF]
                    ks = sin_all[:, st, 0:HALF]
                    nc.vector.tensor_mul(k_rot[:, 0:HALF], pskv[:, 0:HALF], kc)
                    nc.vector.tensor_mul(ktmp, pskv[:, HALF:ROPE], ks)
                    nc.vector.tensor_add(k_rot[:, 0:HALF], k_rot[:, 0:HALF], ktmp)
                    nc.vector.tensor_mul(k_rot[:, HALF:ROPE], pskv[:, HALF:ROPE], kc)
                    nc.vector.tensor_mul(ktmp, pskv[:, 0:HALF], ks)
                    nc.vector.tensor_sub(k_rot[:, HALF:ROPE], k_rot[:, HALF:ROPE], ktmp)
                    nc.vector.tensor_copy(k_rot[:, ROPE:HD], pskv[:, ROPE:HD])

                    # scale in place: q by gain*rstd/sqrt(hd), k by rstd
                    for h in range(G):
                        nc.vector.tensor_scalar_mul(q_rot[:, h, :], q_rot[:, h, :], gsc[:, h:h + 1])
                    nc.vector.tensor_scalar_mul(k_rot, k_rot, rstd[:, 4:5])

                    # transposes into [f, s] layouts; evacuations (fp32->fp16) on ScalarE
                    for h in range(G):
                        ptr = pstr_pool.tile([128, 128], F32)
                        nc.tensor.transpose(ptr, q_rot[:, h, :], ident)
                        nc.scalar.copy(qT_all[:, h, st * 128:(st + 1) * 128], ptr)
                    ptr = pstr_pool.tile([128, 128], F32)
                    nc.tensor.transpose(ptr, k_rot, ident)
                    nc.scalar.copy(kT_all[:, st * 128:(st + 1) * 128], ptr)

            # ======== phase 2+3: attention with interleaved output projection ======
            # Attention is software-pipelined by one k-pair: the PE queue per
            # head is [scores(j), AV(j-1), ones(j-1), scores(j+1), ...] with an
            # out-proj chain of the PREVIOUS chunk dropped in after each scores
            # pair, so the PE always has independent work while the scalar
            # engine runs tanh+exp for the freshly computed pair.
            # The softmax denominator costs the PE one ones128 matmul per PAIR
            # (vector pre-sums p0+p1), accumulated+broadcast in a PSUM bank.
            with (
                tc.tile_pool(name="wp", bufs=1) as wp_pool,
                tc.tile_pool(name="p2s", bufs=4) as p2s,
                tc.tile_pool(name="p2n", bufs=3) as p2n,
                tc.tile_pool(name="p3s", bufs=3) as p3s,
                tc.tile_pool(name="pss", bufs=2, space="PSUM") as pss_pool,
                tc.tile_pool(name="psy", bufs=1, space="PSUM") as psy_pool,
                tc.tile_pool(name="psb", bufs=1, space="PSUM") as psb_pool,
                tc.tile_pool(name="pso", bufs=2, space="PSUM") as pso_pool,
            ):
                wpT_sb = wp_pool.tile([128, G, D], F16)
                nc.sync.dma_start(out=wpT_sb,
                                  in_=wpT.rearrange("p (g j) -> p g j", g=G))

                pending = []  # out-proj (st, jc) chains left to emit

                def emit_outproj(n):
                    for _ in range(min(n, len(pending))):
                        st, jc = pending.pop(0)
                        ps_o = pso_pool.tile([128, 512], F32)
                        for g in range(G):
                            nc.tensor.matmul(
                                ps_o,
                                yT_all[:, g, st * 128:(st + 1) * 128],
                                wpT_sb[:, g, jc * 512:(jc + 1) * 512],
                                start=(g == 0), stop=(g == G - 1),
                            )
                        o_sb = p3s.tile([128, 512], F16)
                        nc.vector.tensor_copy(o_sb, ps_o)
                        nc.sync.dma_start(out=osw[st][jc], in_=o_sb)

                def attention_head(c, g):
                    nkv = 4 * (c + 1)
                    npair = nkv // 2
                    qT_c = qT_all[:, g, c * 512:(c + 1) * 512]
                    ps_y = psy_pool.tile([128, 512], F32)
                    ps_b = psb_pool.tile([128, 512], F32)
                    deferred = None  # (kb0, off, p_tile, psum_tile_of_pairsum)

                    def emit_deferred():
                        kb0, off, p, ps01 = deferred
                        for i in range(2):
                            kb = kb0 + i
                            nc.tensor.matmul(ps_y[:, off:512], v_all[:, kb, :],
                                             p[:, i, off:512],
                                             start=(kb == 0), stop=(kb == nkv - 1))
                        j = kb0 // 2
                        nc.tensor.matmul(ps_b[:, off:512], ones128, ps01[:, off:512],
                                         start=(j == 0), stop=(j == npair - 1))

                    for j, kb0 in enumerate(range(0, nkv, 2)):
                        # the r>=2 diagonal pair only touches sq >= 256
                        off = 256 if kb0 - 4 * c == 2 else 0
                        ps_s = pss_pool.tile([128, 2, 512], F32, tag="ps_s")
                        for i in range(2):
                            kb = kb0 + i
                            nc.tensor.matmul(
                                ps_s[:, i, off:512],
                                kT_all[:, kb * 128:(kb + 1) * 128], qT_c[:, off:512],
                                start=True, stop=True,
                            )
                        emit_outproj(1)
                        if deferred is not None:
                            emit_deferred()
                        t = p2s.tile([128, 2, 512], F32, tag="t")
                        nc.scalar.activation(t[:, :, off:512], ps_s[:, :, off:512],
                                             mybir.ActivationFunctionType.Tanh,
                                             scale=1.0 / CAP)
                        p = p2s.tile([128, 2, 512], BF16, tag="p")
                        nc.scalar.activation(p[:, :, off:512], t[:, :, off:512],
                                             mybir.ActivationFunctionType.Exp,
                                             scale=CAP)
                        for i in range(2):
                            r = kb0 + i - 4 * c
                            if r >= 0:
                                nc.vector.tensor_mul(p[:, i, off:512], p[:, i, off:512],
                                                     masks[:, r, off:512])
                        ps01 = p2s.tile([128, 512], BF16, tag="ps01")
                        nc.vector.tensor_add(ps01[:, off:512], p[:, 0, off:512],
                                             p[:, 1, off:512])
                        deferred = (kb0, off, p, ps01)
                    emit_deferred()
                    # ps_b holds the denominator broadcast to all partitions
                    recip = p2n.tile([128, 512], F32, tag="recip")
                    nc.vector.reciprocal_approx_fast(recip, ps_b)
                    nc.vector.tensor_mul(yT_all[:, g, c * 512:(c + 1) * 512], ps_y, recip)

                for c in range(NCH):
                    for g in range(G):
                        attention_head(c, g)
                    pending.extend((st, jc)
                                   for st in range(4 * c, 4 * c + 4)
                                   for jc in range(4))
                emit_outproj(len(pending))

    nc.compile()
    return nc


def _host_prep(x, Wq, Wk, Wv, Wproj, q_gain):
    inv_freq = 1.0 / (ROPE_BASE ** (np.arange(0, ROPE, 2, dtype=np.float32) / ROPE))
    t = np.arange(S, dtype=np.float32)
    freqs = np.outer(t, inv_freq).astype(np.float32)  # [S, 16]
    cos = np.cos(freqs)
    sin = np.sin(freqs)
    # [S, G*HALF] -> swizzle (t p) f -> [128, NST * G*HALF]
    cos4 = np.tile(cos[:, None, :], (1, G, 1)).reshape(NST, 128, G * HALF)
    sin4 = np.tile(sin[:, None, :], (1, G, 1)).reshape(NST, 128, G * HALF)
    cos4 = np.ascontiguousarray(cos4.transpose(1, 0, 2).reshape(128, NST * G * HALF)).astype(np.float16)
    sin4 = np.ascontiguousarray(sin4.transpose(1, 0, 2).reshape(128, NST * G * HALF)).astype(np.float16)

    # x: [B, S, D] -> xT [D, S] -> [st, p, dt*128] where row d = dt*128 + p,
    # col s = st*128 + s'
    xsw = []
    for b in range(B):
        xT = x[b].T.reshape(NDT, 128, NST, 128)          # [dt, p, st, s']
        xsw.append(np.ascontiguousarray(
            xT.transpose(2, 1, 0, 3).reshape(NST, 128, NDT * 128)).astype(np.float16))

    in_maps = []
    for core in range(8):
        b, h = core // HKV, core % HKV
        wqkv_cat = np.concatenate(
            [Wq[512 * h:512 * h + 512].T,
             Wk[128 * h:128 * h + 128].T,
             Wv[128 * h:128 * h + 128].T], axis=1
        )                                                 # [D, FQKV]
        wqkv_sw = np.ascontiguousarray(
            wqkv_cat.reshape(NDT, 128, FQKV).transpose(1, 0, 2).reshape(128, NDT * FQKV)
        ).astype(np.float16)
        wpT = Wproj[:, 512 * h:512 * h + 512].T           # [512, D]
        wpT_sw = np.ascontiguousarray(
            wpT.reshape(G, 128, D).transpose(1, 0, 2).reshape(128, G * D)
        ).astype(np.float16)
        gains = np.ascontiguousarray(
            np.broadcast_to((q_gain[G * h:G * h + G] / math.sqrt(HD)).astype(np.float32)[None, :],
                            (128, G))
        )
        in_maps.append({
            "xsw": xsw[b],
            "wqkv": wqkv_sw,
            "wpT": wpT_sw,
            "gains": gains,
            "cos4": cos4,
            "sin4": sin4,
        })
    return in_maps


def kernel(x, Wq, Wk, Wv, Wproj, q_gain, _trace=False):
    x = np.asarray(x, dtype=np.float32)
    Wq = np.asarray(Wq, dtype=np.float32)
    Wk = np.asarray(Wk, dtype=np.float32)
    Wv = np.asarray(Wv, dtype=np.float32)
    Wproj = np.asarray(Wproj, dtype=np.float32)
    q_gain = np.asarray(q_gain, dtype=np.float32)

    if "nc" not in _CACHE:
        _CACHE["nc"] = _build()
    nc = _CACHE["nc"]

    in_maps = _host_prep(x, Wq, Wk, Wv, Wproj, q_gain)
    res = run_bass_kernel_spmd(nc, in_maps, core_ids=list(range(8)), trace=_trace)

    out = np.empty((B, S, D), dtype=np.float32)
    for b in range(B):
        acc = np.zeros((NST, NCH, 128, 512), dtype=np.float32)
        for h in range(HKV):
            acc += res.results[b * HKV + h]["osw"]
        # [st, jc, p, n] -> [st*128+p, jc*512+n]
        out[b] = acc.transpose(0, 2, 1, 3).reshape(S, D)
    if _trace:
        return out, res
    return out
